# revision 41
# baseline (speedup 1.0000x reference)
"""Trainium2 Bass kernel for nn_DeepLatent chamfer+BCE loss.

loss = mean_b [ chamfer(est_b, gt_b) + bce(labels_b, labels_est_b) ]

Strategy: pure data parallel over B=32 across 8 cores (4 batches/core).
Per batch the PE produces nd2[n,m] = -(|e_n|^2 + |g_m|^2 - 2 e_n.g_m)
directly via a K=13 split-precision contraction (est side negated in
_pack_inputs), so every on-device reduction is a MAX (required by the
GpSimd cross-lane reduce, which only supports max). K is padded to 32 and
est tiles are spread over the four 32-row groups of the PE array
(tile_position row tiling).

Per [128, 2048] nd2 tile:
  - ScalarE casts PSUM fp32 -> SBUF ACC_DT in two [128,1024] copies
    (the only PSUM drain; PSUM = 3 double-bank chunks + transpose tile)
  - VectorE: one tt_max L1 fold (2x mode) into t4b; tt_max acc2
    accumulate over est tiles (2x mode)
dist1 tail: per half-batch, big 3D-AP tt_max tree over t4b (folded while
later tiles still stream) + one small 1x reduce.
dist2 tail per batch: PE transposes acc2 into one [128,16,128] PSUM tile,
one batched 1x reduce max over the (now free) est axis.
BCE once per core: softplus(z) = ln(1+e^z) on ScalarE with sum
accumulation, minus sum t*z via VectorE scalar_tensor_tensor.

Negated-max formulation notes: DVE reduce ops and accumulator-bearing
instructions run at 1x only (the accumulator disables 2x/4x modes), so
all O(N^2) reductions are expressed as 2x-mode tensor_tensor folds; the
negation itself costs nothing (host flips signs in est13 packing and
un-negates the stats columns).

Per-core output: [128, 2*BPC+1] partial sums. Chamfer columns hold
sum(min(M,0)) = -sum(relu(-M)) of the negated mins; the host negates,
finishes the 128-way partition sum and the mean over samples.
"""

import os
import numpy as np

B, N = 32, 2048
NCORES = 8
BPC = B // NCORES  # batches per core
NTILES = N // 128  # 16 est tiles per batch
OUTW = BPC + 1  # per-batch combined chamfer sums + one bce column

ACC_DT_STR = os.environ.get("CHAMFER_ACC_DT", "bfloat16")

_cache = {}


def _build_program():
    import sys
    if "/opt/trn_rl_repo" not in sys.path:
        sys.path.insert(0, "/opt/trn_rl_repo")
    import concourse.bass as bass
    import concourse.tile as tile
    from concourse import bacc, mybir

    ACC_DT = getattr(mybir.dt, ACC_DT_STR)
    FP32 = mybir.dt.float32
    AOP = mybir.AluOpType
    AFT = mybir.ActivationFunctionType

    nc = bacc.Bacc("TRN2", target_bir_lowering=False, debug=False)

    estP_d = nc.dram_tensor("estP", [128, BPC * 512], ACC_DT, kind="ExternalInput")
    gtP_d = nc.dram_tensor("gtP", [128, BPC * 2048], ACC_DT, kind="ExternalInput")
    z_d = nc.dram_tensor("zt", [128, BPC * 16], FP32, kind="ExternalInput")
    t_d = nc.dram_tensor("tt", [128, BPC * 16], FP32, kind="ExternalInput")
    id_d = nc.dram_tensor("ident", [128, 128], ACC_DT, kind="ExternalInput")
    out_d = nc.dram_tensor("out", [128, OUTW], FP32, kind="ExternalOutput")

    with tile.TileContext(nc) as tc:
        with (
            tc.tile_pool(name="const", bufs=1) as cpool,
            tc.tile_pool(name="acc2", bufs=2) as acc2_pool,
            tc.tile_pool(name="rowc", bufs=4) as rowc_pool,
            tc.tile_pool(name="mins", bufs=2) as mins_pool,
            tc.tile_pool(name="da", bufs=2) as da_pool,
            tc.tile_pool(name="stats", bufs=1) as stats_pool,
            tc.tile_pool(name="ps", bufs=3, space=bass.MemorySpace.PSUM) as ps_pool,
            tc.tile_pool(name="tp", bufs=1, space=bass.MemorySpace.PSUM) as tp_pool,
        ):
            # ---- load inputs; batch-0 chunks first so compute starts ASAP
            est_sb = cpool.tile([128, BPC * 512], ACC_DT, tag="est")
            gt_sb = cpool.tile([128, BPC * 2048], ACC_DT, tag="gt")
            z_sb = cpool.tile([128, BPC * 16], FP32, tag="z")
            t_sb = cpool.tile([128, BPC * 16], FP32, tag="t")
            id_sb = cpool.tile([128, 128], ACC_DT, tag="id")

            # batch-0 data split into small pieces across many DMA queues
            first_cols = []
            for p in range(4):  # gt batch-0 in 256-col (64KB) pieces
                first_cols.append(("gt", 256 * p, 256 * (p + 1)))
            first_cols.append(("est", 0, 256))
            first_cols.append(("est", 256, 512))
            for kind, c0, c1 in first_cols:
                sb, dr = (gt_sb, gtP_d) if kind == "gt" else (est_sb, estP_d)
                nc.sync.dma_start(sb[:, c0:c1], dr[:, c0:c1])
            nc.sync.dma_start(gt_sb[:, 1024:1536], gtP_d[:, 1024:1536])
            nc.sync.dma_start(gt_sb[:, 1536:2048], gtP_d[:, 1536:2048])
            nc.sync.dma_start(id_sb[:], id_d[:])
            nc.sync.dma_start(est_sb[:, 512:], estP_d[:, 512:])
            for bb in range(1, BPC):
                nc.sync.dma_start(gt_sb[:, 2048 * bb:2048 * (bb + 1)],
                                  gtP_d[:, 2048 * bb:2048 * (bb + 1)])
            nc.sync.dma_start(z_sb[:], z_d[:])
            nc.sync.dma_start(t_sb[:], t_d[:])

            # tiny PE ops that absorb each DMA-completion wait into PE's
            # vector clock (walrus allows only ONE sync wait on a matmul)
            # warms ordered so the pieces gating tile (b=0, i=0) come first
            est_warm = [0, 256]
            gt_warm = [0, 256, 512, 768, 1024, 1536]
            late_est_warm = [512]
            late_gt_warm = [2048, 4096, 6144]
            nw = len(est_warm) + len(gt_warm) + len(late_est_warm) + \
                len(late_gt_warm)
            warm = tp_pool.tile([1, nw], FP32, tag="tp")
            wk = 0

            def _warm(sb, col):
                nonlocal wk
                nc.tensor.matmul(
                    warm[0:1, wk:wk + 1], sb[0:32, col:col + 1],
                    sb[0:32, col:col + 1], start=True, stop=True,
                )
                wk += 1

            for col in est_warm:
                _warm(est_sb, col)
            for col in gt_warm:
                _warm(gt_sb, col)

            stats = stats_pool.tile([128, OUTW], FP32)

            for b in range(BPC):
                acc2 = acc2_pool.tile([128, 2048], ACC_DT)
                # combined per-batch chamfer partials: cols 0:16 dist1,
                # 16:32 dist2, summed by a single tensor_scalar
                minsB = mins_pool.tile([128, 2 * NTILES], FP32, tag="mB")

                # per-i dist1 L1 folds land here; tree-reduced once per
                # batch with big 3D-AP instructions (fewer DVE bubbles)
                t4b = mins_pool.tile([128, NTILES, 1024], ACC_DT, tag="t4b")
                for i in range(NTILES):
                    a, c = i % 4, i // 4
                    lhsT = est_sb[32 * a:32 * a + 32,
                                  b * 512 + 128 * c: b * 512 + 128 * (c + 1)]
                    rowc = acc2 if i == 0 else rowc_pool.tile([128, 2048], ACC_DT)
                    if b == 0 and i == 0:
                        # first tile: quarter-size casts in pair order so
                        # the DVE's first folds start ~1us sooner
                        for q in (0, 2, 1, 3):
                            ps = ps_pool.tile([128, 512], FP32)
                            m0 = 512 * q
                            nc.tensor.matmul(
                                ps[:], lhsT,
                                gt_sb[32 * a:32 * a + 32, m0:m0 + 512],
                                start=True, stop=True,
                                tile_position=(32 * a, 0),
                            )
                            nc.scalar.copy(rowc[:, m0:m0 + 512], ps[:])
                            if q in (2, 3):
                                j0 = 512 * (q - 2)
                                nc.vector.tensor_tensor(
                                    t4b[:, 0, j0:j0 + 512],
                                    rowc[:, j0:j0 + 512],
                                    rowc[:, j0 + 1024:j0 + 1536],
                                    op=AOP.max)
                        continue
                    for h in range(2):
                        ps = ps_pool.tile([128, 1024], FP32)
                        for jj in range(2):
                            m0 = b * 2048 + h * 1024 + jj * 512
                            nc.tensor.matmul(
                                ps[:, jj * 512:(jj + 1) * 512],
                                lhsT,
                                gt_sb[32 * a:32 * a + 32, m0:m0 + 512],
                                start=True, stop=True,
                                tile_position=(32 * a, 0),
                            )
                        # cast to SBUF working dtype on ScalarE
                        nc.scalar.copy(rowc[:, h * 1024:(h + 1) * 1024], ps[:])
                    # dist1 level-1 fold of the tt_max tree (2x mode)
                    nc.vector.tensor_tensor(
                        t4b[:, i, :], rowc[:, :1024], rowc[:, 1024:],
                        op=AOP.max)
                    # dist2 accumulator: elementwise max over est tiles
                    # (i == 0 wrote the cast directly into acc2)
                    if i > 0:
                        nc.vector.tensor_tensor(
                            acc2[:], rowc[:], acc2[:], op=AOP.max,
                        )
                    if b == 0 and i == 1:
                        # late DMA-wait absorbers: regions only needed by
                        # later tiles; issued after tile 0 so they don't
                        # delay the first casts
                        for col in late_est_warm:
                            _warm(est_sb, col)
                        for col in late_gt_warm:
                            _warm(gt_sb, col)
                        warm2 = tp_pool.tile([128, 128], ACC_DT, tag="tp")
                        nc.tensor.transpose(warm2[:], id_sb[:], id_sb[:])
                    # batched dist1 tail: fold finished tile groups while
                    # later tiles still stream; the last batch folds its
                    # back half in quarters to shrink the end-of-kernel
                    # serial chain
                    last = b == BPC - 1
                    folds = (7, 11, 15) if last else (7, 15)
                    if i in folds:
                        if i == 7:
                            lo, n = 0, 8
                        elif last:
                            lo, n = 4 * (i // 4), 4
                        else:
                            lo, n = 8, 8
                        w = 1024
                        while w > 8:
                            nc.vector.tensor_tensor(
                                t4b[:, lo:lo + n, :w // 2],
                                t4b[:, lo:lo + n, :w // 2],
                                t4b[:, lo:lo + n, w // 2:w], op=AOP.max)
                            w //= 2
                nc.vector.tensor_reduce(
                    minsB[:, :NTILES], t4b[:, :, :8],
                    axis=mybir.AxisListType.X, op=AOP.max)

                # dist2 tail: transpose acc2 in 128-col strips on the PE
                # into one PSUM tile, one batched reduce max over the (now
                # free) est-point axis
                tp = tp_pool.tile([128, NTILES, 128], ACC_DT, tag="tp")
                for u in range(NTILES):
                    nc.tensor.transpose(
                        tp[:, u, :],
                        acc2[:, 128 * u:128 * (u + 1)],
                        id_sb[:],
                    )
                nc.vector.tensor_reduce(
                    minsB[:, NTILES:], tp[:],
                    axis=mybir.AxisListType.X, op=AOP.max,
                )
                # stats: M = max(-d2); sum(min(M,0)) = -sum(relu(min d2))
                # over both chamfer directions at once; host negates.
                mBr = da_pool.tile([128, 2 * NTILES], FP32, tag="mBr")
                nc.vector.tensor_scalar(
                    out=mBr[:], in0=minsB[:], scalar1=0.0, scalar2=None,
                    op0=AOP.min, op1=AOP.add,
                    accum_out=stats[:, b:b + 1],
                )

            # bce for all batches at once:
            #   sum softplus(z) - sum t*z, softplus(z) = ln(1 + e^z)
            # (z ~ N(0,1): |z| < 6, so e^z cannot overflow)
            sp = da_pool.tile([128, BPC * 16], FP32, tag="sp")
            spa = da_pool.tile([128, 1], FP32, tag="spa")
            tza = da_pool.tile([128, 1], FP32, tag="tza")
            nc.scalar.activation(sp[:], z_sb[:], AFT.Exp)
            nc.scalar.activation(sp[:], sp[:], AFT.Ln, bias=1.0, accum_out=spa[:])
            tzj = da_pool.tile([128, BPC * 16], FP32, tag="tzj")
            nc.vector.scalar_tensor_tensor(
                out=tzj[:], in0=z_sb[:], scalar=1.0, in1=t_sb[:],
                op0=AOP.mult, op1=AOP.mult, accum_out=tza[:],
            )
            nc.vector.tensor_tensor(
                stats[:, BPC:BPC + 1], spa[:], tza[:],
                op=AOP.subtract,
            )

            # per-partition partial sums go to the host, which finishes
            # the 128-way partition sum (4.5KB, negligible)
            nc.sync.dma_start(out_d[:], stats[:])

    nc.compile()
    return nc


def _pack_inputs(obs_est, obs_gt, labels_est, labels):
    """Build per-core input maps (host-side layout prep only)."""
    obs_est = np.ascontiguousarray(obs_est, dtype=np.float32)
    obs_gt = np.ascontiguousarray(obs_gt, dtype=np.float32)
    labels_est = np.ascontiguousarray(labels_est, dtype=np.float32)
    labels = np.ascontiguousarray(labels, dtype=np.float32)

    import ml_dtypes
    BF = ml_dtypes.bfloat16 if ACC_DT_STR == "bfloat16" else np.float32

    def split(v):
        hi = v.astype(ml_dtypes.bfloat16).astype(np.float32)
        lo = v - hi
        return hi, lo

    # split-precision operands: d2 = x2 + y2 - 2 e.g with
    #   x2,y2 as bf16 hi+lo pairs (exact to ~2^-16)
    #   e.g  as ehi*ghi + ehi*glo + elo*ghi (products exact in fp32 PSUM)
    # The est side is NEGATED so the device computes -d2 (all reductions
    # become max, as required by the GpSimd cross-lane reduce).
    x2 = (obs_est ** 2).sum(-1)  # [B, N]
    y2 = (obs_gt ** 2).sum(-1)
    one = np.ones_like(x2)
    x2h, x2l = split(x2)
    y2h, y2l = split(y2)
    eh, el = split(obs_est)  # [B, N, 3]
    gh, gl = split(obs_gt)
    NK = 13
    est13 = -np.stack(
        [x2h, x2l, one, one,
         -2 * eh[..., 0], -2 * eh[..., 1], -2 * eh[..., 2],
         -2 * eh[..., 0], -2 * eh[..., 1], -2 * eh[..., 2],
         -2 * el[..., 0], -2 * el[..., 1], -2 * el[..., 2]], axis=1
    )  # [B, 13, N]
    gt13 = np.stack(
        [one, one, y2h, y2l,
         gh[..., 0], gh[..., 1], gh[..., 2],
         gl[..., 0], gl[..., 1], gl[..., 2],
         gh[..., 0], gh[..., 1], gh[..., 2]], axis=1
    )  # [B, 13, N]

    # estP[b, 32a+k, 128c+p] = est13[b, k, (4c+a)*128+p]; rows 13..31 zero
    estP = np.zeros((B, 128, 512), BF)
    est13_t = est13.reshape(B, NK, NTILES, 128)
    for i in range(NTILES):
        a, c = i % 4, i // 4
        estP[:, 32 * a:32 * a + NK, 128 * c:128 * (c + 1)] = est13_t[:, :, i, :]

    # gtP[b, 32a+k, m] = gt13[b, k, m], replicated over the 4 row groups
    gtP = np.zeros((B, 128, 2048), BF)
    for a in range(4):
        gtP[:, 32 * a:32 * a + NK, :] = gt13

    in_maps = []
    for core in range(NCORES):
        bs = slice(core * BPC, (core + 1) * BPC)
        # [BPC,128,X] -> [128, BPC*X] column blocks per batch
        e = estP[bs].transpose(1, 0, 2).reshape(128, BPC * 512)
        g = gtP[bs].transpose(1, 0, 2).reshape(128, BPC * 2048)
        z = labels_est[bs].reshape(BPC, 128, 16).transpose(1, 0, 2).reshape(
            128, BPC * 16)
        t = labels[bs].reshape(BPC, 128, 16).transpose(1, 0, 2).reshape(
            128, BPC * 16)
        in_maps.append({
            "estP": np.ascontiguousarray(e),
            "gtP": np.ascontiguousarray(g),
            "zt": np.ascontiguousarray(z),
            "tt": np.ascontiguousarray(t),
            "ident": np.eye(128, dtype=BF),
        })
    return in_maps


def kernel(obs_est, obs_gt, labels_est, labels):
    import sys
    if "/opt/trn_rl_repo" not in sys.path:
        sys.path.insert(0, "/opt/trn_rl_repo")
    from concourse import bass_utils

    if "nc" not in _cache:
        _cache["nc"] = _build_program()
    nc = _cache["nc"]

    in_maps = _pack_inputs(obs_est, obs_gt, labels_est, labels)

    trace = bool(int(os.environ.get("CHAMFER_TRACE", "0")))
    res = bass_utils.run_bass_kernel_spmd(
        nc, in_maps, core_ids=list(range(NCORES)), trace=trace
    )
    _cache["last_result"] = res

    # out: [128, OUTW]; cols 2b / 2b+1 hold NEGATED per-batch dist1/dist2
    # relu'd sums; col 2*BPC = sum over batches of (sum softplus - sum t*z)
    total = 0.0
    for c in range(NCORES):
        s = np.asarray(res.results[c]["out"]).astype(np.float64).sum(axis=0)
        total += -s[:BPC].sum() + s[BPC]
    return np.float32(total / (N * B))


# revision 42
# speedup vs baseline: 1.0026x; 1.0026x over previous
"""Trainium2 Bass kernel for nn_DeepLatent chamfer+BCE loss.

loss = mean_b [ chamfer(est_b, gt_b) + bce(labels_b, labels_est_b) ]

Strategy: pure data parallel over B=32 across 8 cores (4 batches/core).
Per batch the PE produces nd2[n,m] = -(|e_n|^2 + |g_m|^2 - 2 e_n.g_m)
directly via a K=13 split-precision contraction (est side negated in
_pack_inputs), so every on-device reduction is a MAX (required by the
GpSimd cross-lane reduce, which only supports max). K is padded to 32 and
est tiles are spread over the four 32-row groups of the PE array
(tile_position row tiling).

Per [128, 2048] nd2 tile:
  - ScalarE casts PSUM fp32 -> SBUF ACC_DT in two [128,1024] copies
    (the only PSUM drain; PSUM = 3 double-bank chunks + transpose tile)
  - VectorE: one tt_max L1 fold (2x mode) into t4b; tt_max acc2
    accumulate over est tiles (2x mode)
dist1 tail: per half-batch, big 3D-AP tt_max tree over t4b (folded while
later tiles still stream) + one small 1x reduce.
dist2 tail per batch: PE transposes acc2 into one [128,16,128] PSUM tile,
one batched 1x reduce max over the (now free) est axis.
BCE once per core: softplus(z) = ln(1+e^z) on ScalarE with sum
accumulation, minus sum t*z via VectorE scalar_tensor_tensor.

Negated-max formulation notes: DVE reduce ops and accumulator-bearing
instructions run at 1x only (the accumulator disables 2x/4x modes), so
all O(N^2) reductions are expressed as 2x-mode tensor_tensor folds; the
negation itself costs nothing (host flips signs in est13 packing and
un-negates the stats columns).

Per-core output: [128, 2*BPC+1] partial sums. Chamfer columns hold
sum(min(M,0)) = -sum(relu(-M)) of the negated mins; the host negates,
finishes the 128-way partition sum and the mean over samples.
"""

import os
import numpy as np

B, N = 32, 2048
NCORES = 8
BPC = B // NCORES  # batches per core
NTILES = N // 128  # 16 est tiles per batch
OUTW = BPC + 1  # per-batch combined chamfer sums + one bce column

ACC_DT_STR = os.environ.get("CHAMFER_ACC_DT", "bfloat16")

_cache = {}


def _build_program():
    import sys
    if "/opt/trn_rl_repo" not in sys.path:
        sys.path.insert(0, "/opt/trn_rl_repo")
    import concourse.bass as bass
    import concourse.tile as tile
    from concourse import bacc, mybir

    ACC_DT = getattr(mybir.dt, ACC_DT_STR)
    FP32 = mybir.dt.float32
    AOP = mybir.AluOpType
    AFT = mybir.ActivationFunctionType

    nc = bacc.Bacc("TRN2", target_bir_lowering=False, debug=False)

    estP_d = nc.dram_tensor("estP", [128, BPC * 512], ACC_DT, kind="ExternalInput")
    gtP_d = nc.dram_tensor("gtP", [128, BPC * 2048], ACC_DT, kind="ExternalInput")
    z_d = nc.dram_tensor("zt", [128, BPC * 16], FP32, kind="ExternalInput")
    t_d = nc.dram_tensor("tt", [128, BPC * 16], FP32, kind="ExternalInput")
    id_d = nc.dram_tensor("ident", [128, 128], ACC_DT, kind="ExternalInput")
    out_d = nc.dram_tensor("out", [128, OUTW], FP32, kind="ExternalOutput")

    with tile.TileContext(nc) as tc:
        with (
            tc.tile_pool(name="const", bufs=1) as cpool,
            tc.tile_pool(name="acc2", bufs=2) as acc2_pool,
            tc.tile_pool(name="rowc", bufs=4) as rowc_pool,
            tc.tile_pool(name="mins", bufs=2) as mins_pool,
            tc.tile_pool(name="da", bufs=2) as da_pool,
            tc.tile_pool(name="stats", bufs=1) as stats_pool,
            tc.tile_pool(name="ps", bufs=3, space=bass.MemorySpace.PSUM) as ps_pool,
            tc.tile_pool(name="tp", bufs=1, space=bass.MemorySpace.PSUM) as tp_pool,
        ):
            # ---- load inputs; batch-0 chunks first so compute starts ASAP
            est_sb = cpool.tile([128, BPC * 512], ACC_DT, tag="est")
            gt_sb = cpool.tile([128, BPC * 2048], ACC_DT, tag="gt")
            z_sb = cpool.tile([128, BPC * 16], FP32, tag="z")
            t_sb = cpool.tile([128, BPC * 16], FP32, tag="t")
            id_sb = cpool.tile([128, 128], ACC_DT, tag="id")

            # batch-0 data split into small pieces across many DMA queues
            first_cols = []
            for p in range(4):  # gt batch-0 in 256-col (64KB) pieces
                first_cols.append(("gt", 256 * p, 256 * (p + 1)))
            first_cols.append(("est", 0, 256))
            first_cols.append(("est", 256, 512))
            for kind, c0, c1 in first_cols:
                sb, dr = (gt_sb, gtP_d) if kind == "gt" else (est_sb, estP_d)
                nc.sync.dma_start(sb[:, c0:c1], dr[:, c0:c1])
            nc.sync.dma_start(gt_sb[:, 1024:1536], gtP_d[:, 1024:1536])
            nc.sync.dma_start(gt_sb[:, 1536:2048], gtP_d[:, 1536:2048])
            nc.sync.dma_start(id_sb[:], id_d[:])
            nc.sync.dma_start(est_sb[:, 512:], estP_d[:, 512:])
            for bb in range(1, BPC):
                nc.sync.dma_start(gt_sb[:, 2048 * bb:2048 * (bb + 1)],
                                  gtP_d[:, 2048 * bb:2048 * (bb + 1)])
            nc.sync.dma_start(z_sb[:], z_d[:])
            nc.sync.dma_start(t_sb[:], t_d[:])

            # tiny PE ops that absorb each DMA-completion wait into PE's
            # vector clock (walrus allows only ONE sync wait on a matmul)
            # warms ordered so the pieces gating tile (b=0, i=0) come first
            est_warm = [0, 256]
            gt_warm = [0, 256, 512, 768, 1024, 1536]
            late_est_warm = [512]
            late_gt_warm = [2048, 4096, 6144]
            nw = len(est_warm) + len(gt_warm) + len(late_est_warm) + \
                len(late_gt_warm)
            warm = tp_pool.tile([1, nw], FP32, tag="tp")
            wk = 0

            def _warm(sb, col):
                nonlocal wk
                nc.tensor.matmul(
                    warm[0:1, wk:wk + 1], sb[0:32, col:col + 1],
                    sb[0:32, col:col + 1], start=True, stop=True,
                )
                wk += 1

            for col in est_warm:
                _warm(est_sb, col)
            for col in gt_warm:
                _warm(gt_sb, col)

            stats = stats_pool.tile([128, OUTW], FP32)

            for b in range(BPC):
                acc2 = acc2_pool.tile([128, 2048], ACC_DT)
                # combined per-batch chamfer partials: cols 0:16 dist1,
                # 16:32 dist2, summed by a single tensor_scalar
                minsB = mins_pool.tile([128, 2 * NTILES], FP32, tag="mB")

                # per-i dist1 L1 folds land here; tree-reduced once per
                # batch with big 3D-AP instructions (fewer DVE bubbles)
                t4b = mins_pool.tile([128, NTILES, 1024], ACC_DT, tag="t4b")
                for i in range(NTILES):
                    a, c = i % 4, i // 4
                    lhsT = est_sb[32 * a:32 * a + 32,
                                  b * 512 + 128 * c: b * 512 + 128 * (c + 1)]
                    rowc = acc2 if i == 0 else rowc_pool.tile([128, 2048], ACC_DT)
                    if b == 0 and i == 0:
                        # first tile: quarter-size casts in pair order so
                        # the DVE's first folds start ~1us sooner
                        for q in (0, 2, 1, 3):
                            ps = ps_pool.tile([128, 512], FP32)
                            m0 = 512 * q
                            nc.tensor.matmul(
                                ps[:], lhsT,
                                gt_sb[32 * a:32 * a + 32, m0:m0 + 512],
                                start=True, stop=True,
                                tile_position=(32 * a, 0),
                            )
                            nc.scalar.copy(rowc[:, m0:m0 + 512], ps[:])
                            if q in (2, 3):
                                j0 = 512 * (q - 2)
                                nc.vector.tensor_tensor(
                                    t4b[:, 0, j0:j0 + 512],
                                    rowc[:, j0:j0 + 512],
                                    rowc[:, j0 + 1024:j0 + 1536],
                                    op=AOP.max)
                        continue
                    for h in range(2):
                        ps = ps_pool.tile([128, 1024], FP32)
                        for jj in range(2):
                            m0 = b * 2048 + h * 1024 + jj * 512
                            nc.tensor.matmul(
                                ps[:, jj * 512:(jj + 1) * 512],
                                lhsT,
                                gt_sb[32 * a:32 * a + 32, m0:m0 + 512],
                                start=True, stop=True,
                                tile_position=(32 * a, 0),
                            )
                        # cast to SBUF working dtype on ScalarE
                        nc.scalar.copy(rowc[:, h * 1024:(h + 1) * 1024], ps[:])
                    # dist1 level-1 fold of the tt_max tree (2x mode)
                    nc.vector.tensor_tensor(
                        t4b[:, i, :], rowc[:, :1024], rowc[:, 1024:],
                        op=AOP.max)
                    # dist2 accumulator: elementwise max over est tiles
                    # (i == 0 wrote the cast directly into acc2)
                    if i > 0:
                        nc.vector.tensor_tensor(
                            acc2[:], rowc[:], acc2[:], op=AOP.max,
                        )
                    if b == 0 and i == 1:
                        # late DMA-wait absorbers: regions only needed by
                        # later tiles; issued after tile 0 so they don't
                        # delay the first casts
                        for col in late_est_warm:
                            _warm(est_sb, col)
                        for col in late_gt_warm:
                            _warm(gt_sb, col)
                        warm2 = tp_pool.tile([128, 128], ACC_DT, tag="tp")
                        nc.tensor.transpose(warm2[:], id_sb[:], id_sb[:])
                    # batched dist1 tail in halves: fold tiles 0-7 while
                    # tiles 8-15 are still streaming, rest at batch end
                    if i in (7, 15):
                        lo = 8 * (i // 8)
                        w = 1024
                        while w > 8:
                            nc.vector.tensor_tensor(
                                t4b[:, lo:lo + 8, :w // 2],
                                t4b[:, lo:lo + 8, :w // 2],
                                t4b[:, lo:lo + 8, w // 2:w], op=AOP.max)
                            w //= 2
                nc.vector.tensor_reduce(
                    minsB[:, :NTILES], t4b[:, :, :8],
                    axis=mybir.AxisListType.X, op=AOP.max)

                # dist2 tail: transpose acc2 in 128-col strips on the PE
                # into one PSUM tile, one batched reduce max over the (now
                # free) est-point axis
                tp = tp_pool.tile([128, NTILES, 128], ACC_DT, tag="tp")
                for u in range(NTILES):
                    nc.tensor.transpose(
                        tp[:, u, :],
                        acc2[:, 128 * u:128 * (u + 1)],
                        id_sb[:],
                    )
                nc.vector.tensor_reduce(
                    minsB[:, NTILES:], tp[:],
                    axis=mybir.AxisListType.X, op=AOP.max,
                )
                # stats: M = max(-d2); sum(min(M,0)) = -sum(relu(min d2))
                # over both chamfer directions at once; host negates.
                mBr = da_pool.tile([128, 2 * NTILES], FP32, tag="mBr")
                nc.vector.tensor_scalar(
                    out=mBr[:], in0=minsB[:], scalar1=0.0, scalar2=None,
                    op0=AOP.min, op1=AOP.add,
                    accum_out=stats[:, b:b + 1],
                )

            # bce for all batches at once:
            #   sum softplus(z) - sum t*z, softplus(z) = ln(1 + e^z)
            # (z ~ N(0,1): |z| < 6, so e^z cannot overflow)
            sp = da_pool.tile([128, BPC * 16], FP32, tag="sp")
            spa = da_pool.tile([128, 1], FP32, tag="spa")
            tza = da_pool.tile([128, 1], FP32, tag="tza")
            nc.scalar.activation(sp[:], z_sb[:], AFT.Exp)
            nc.scalar.activation(sp[:], sp[:], AFT.Ln, bias=1.0, accum_out=spa[:])
            tzj = da_pool.tile([128, BPC * 16], FP32, tag="tzj")
            nc.vector.scalar_tensor_tensor(
                out=tzj[:], in0=z_sb[:], scalar=1.0, in1=t_sb[:],
                op0=AOP.mult, op1=AOP.mult, accum_out=tza[:],
            )
            nc.vector.tensor_tensor(
                stats[:, BPC:BPC + 1], spa[:], tza[:],
                op=AOP.subtract,
            )

            # per-partition partial sums go to the host, which finishes
            # the 128-way partition sum (4.5KB, negligible)
            nc.sync.dma_start(out_d[:], stats[:])

    nc.compile()
    return nc


def _pack_inputs(obs_est, obs_gt, labels_est, labels):
    """Build per-core input maps (host-side layout prep only)."""
    obs_est = np.ascontiguousarray(obs_est, dtype=np.float32)
    obs_gt = np.ascontiguousarray(obs_gt, dtype=np.float32)
    labels_est = np.ascontiguousarray(labels_est, dtype=np.float32)
    labels = np.ascontiguousarray(labels, dtype=np.float32)

    import ml_dtypes
    BF = ml_dtypes.bfloat16 if ACC_DT_STR == "bfloat16" else np.float32

    def split(v):
        hi = v.astype(ml_dtypes.bfloat16).astype(np.float32)
        lo = v - hi
        return hi, lo

    # split-precision operands: d2 = x2 + y2 - 2 e.g with
    #   x2,y2 as bf16 hi+lo pairs (exact to ~2^-16)
    #   e.g  as ehi*ghi + ehi*glo + elo*ghi (products exact in fp32 PSUM)
    # The est side is NEGATED so the device computes -d2 (all reductions
    # become max, as required by the GpSimd cross-lane reduce).
    x2 = (obs_est ** 2).sum(-1)  # [B, N]
    y2 = (obs_gt ** 2).sum(-1)
    one = np.ones_like(x2)
    x2h, x2l = split(x2)
    y2h, y2l = split(y2)
    eh, el = split(obs_est)  # [B, N, 3]
    gh, gl = split(obs_gt)
    NK = 13
    est13 = -np.stack(
        [x2h, x2l, one, one,
         -2 * eh[..., 0], -2 * eh[..., 1], -2 * eh[..., 2],
         -2 * eh[..., 0], -2 * eh[..., 1], -2 * eh[..., 2],
         -2 * el[..., 0], -2 * el[..., 1], -2 * el[..., 2]], axis=1
    )  # [B, 13, N]
    gt13 = np.stack(
        [one, one, y2h, y2l,
         gh[..., 0], gh[..., 1], gh[..., 2],
         gl[..., 0], gl[..., 1], gl[..., 2],
         gh[..., 0], gh[..., 1], gh[..., 2]], axis=1
    )  # [B, 13, N]

    # estP[b, 32a+k, 128c+p] = est13[b, k, (4c+a)*128+p]; rows 13..31 zero
    estP = np.zeros((B, 128, 512), BF)
    est13_t = est13.reshape(B, NK, NTILES, 128)
    for i in range(NTILES):
        a, c = i % 4, i // 4
        estP[:, 32 * a:32 * a + NK, 128 * c:128 * (c + 1)] = est13_t[:, :, i, :]

    # gtP[b, 32a+k, m] = gt13[b, k, m], replicated over the 4 row groups
    gtP = np.zeros((B, 128, 2048), BF)
    for a in range(4):
        gtP[:, 32 * a:32 * a + NK, :] = gt13

    in_maps = []
    for core in range(NCORES):
        bs = slice(core * BPC, (core + 1) * BPC)
        # [BPC,128,X] -> [128, BPC*X] column blocks per batch
        e = estP[bs].transpose(1, 0, 2).reshape(128, BPC * 512)
        g = gtP[bs].transpose(1, 0, 2).reshape(128, BPC * 2048)
        z = labels_est[bs].reshape(BPC, 128, 16).transpose(1, 0, 2).reshape(
            128, BPC * 16)
        t = labels[bs].reshape(BPC, 128, 16).transpose(1, 0, 2).reshape(
            128, BPC * 16)
        in_maps.append({
            "estP": np.ascontiguousarray(e),
            "gtP": np.ascontiguousarray(g),
            "zt": np.ascontiguousarray(z),
            "tt": np.ascontiguousarray(t),
            "ident": np.eye(128, dtype=BF),
        })
    return in_maps


def kernel(obs_est, obs_gt, labels_est, labels):
    import sys
    if "/opt/trn_rl_repo" not in sys.path:
        sys.path.insert(0, "/opt/trn_rl_repo")
    from concourse import bass_utils

    if "nc" not in _cache:
        _cache["nc"] = _build_program()
    nc = _cache["nc"]

    in_maps = _pack_inputs(obs_est, obs_gt, labels_est, labels)

    trace = bool(int(os.environ.get("CHAMFER_TRACE", "0")))
    res = bass_utils.run_bass_kernel_spmd(
        nc, in_maps, core_ids=list(range(NCORES)), trace=trace
    )
    _cache["last_result"] = res

    # out: [128, OUTW]; cols 2b / 2b+1 hold NEGATED per-batch dist1/dist2
    # relu'd sums; col 2*BPC = sum over batches of (sum softplus - sum t*z)
    total = 0.0
    for c in range(NCORES):
        s = np.asarray(res.results[c]["out"]).astype(np.float64).sum(axis=0)
        total += -s[:BPC].sum() + s[BPC]
    return np.float32(total / (N * B))
